# revision 10
# baseline (speedup 1.0000x reference)
"""MoE routing kernel for Trainium2 (8 NeuronCores, SPMD data-parallel).

Problem: T=16384 tokens, D=1024, E=8 experts, top-2 routing.
  gate_logits = x @ Wg.T + bg ; top2 -> softmax -> combine
  out[t] = sum_e combine[t,e] * (x[t] @ We[e].T + be[e])

Sharding: data-parallel. Each core owns 2048 tokens, experts replicated.
Host only slices/transposes inputs and concatenates the 8 output shards.

Device pipeline (per core):
  1. x shard -> SBUF, split into bf16 (hi, lo) planes.
  2. transpose-gather (identity indices) -> xT_hi / xT_lo  [d-part, tok].
  3. fp32-accurate gate logits via 4-term split-bf16 matmul, +bg.
  4. top-2 + softmax on DVE (max8 / max_index), combine weights.
  5. dense per-expert matmul (bf16) with combine-weighted accumulation,
     bias handled as init acc = combine @ be.
"""

import os
import sys
import numpy as np

sys.path.insert(0, "/opt/trn_rl_repo")

import concourse.bass as bass
import concourse.tile as tile
from concourse import bacc, mybir
from concourse.bass_utils import run_bass_kernel_spmd

F32 = mybir.dt.float32
BF16 = mybir.dt.bfloat16
I16 = mybir.dt.int16
U32 = mybir.dt.uint32
ALU = mybir.AluOpType
ACT_EXP = mybir.ActivationFunctionType.Exp

T_FULL = 16384
D = 1024
E = 8
TOPK = 2
NCORES = 8
TC = T_FULL // NCORES     # 2048 tokens per core
P = 128
NT = TC // P              # 16 token tiles
KT = D // P               # 8 contraction tiles
NQ = D // 512             # 2 output column halves


def wrap16(vals: np.ndarray) -> np.ndarray:
    """Pack a 1-D slot array into the [128, n/16] 'wrapped in 16 partitions,
    replicated across 8 cores' SBUF layout used by dma_gather index inputs."""
    n = vals.shape[0]
    assert n % 16 == 0
    w = vals.reshape(n // 16, 16).T        # [16, n/16]
    return np.tile(w, (8, 1)).copy()       # [128, n/16]


def transpose_blocks(nc, out_ap, in_ap, rows, cols):
    """out[c, r] = in[r, c] via DVE 32x32 block transposes.

    in_ap indexable as [rows, cols], out_ap as [cols, rows]; both SBUF,
    rows/cols multiples of 32. Slicing must yield [32, 32] APs.
    """
    for rb in range(rows // 32):
        for cb in range(cols // 32):
            nc.vector.transpose(
                out_ap[cb * 32 : (cb + 1) * 32, rb * 32 : (rb + 1) * 32],
                in_ap[rb * 32 : (rb + 1) * 32, cb * 32 : (cb + 1) * 32],
            )


def build_program():
    nc = bacc.Bacc(
        "TRN2", target_bir_lowering=False, debug=False, num_devices=NCORES
    )

    x_d = nc.dram_tensor("x", [TC, D], F32, kind="ExternalInput").ap()
    wgt_d = nc.dram_tensor("wgt", [D, E], F32, kind="ExternalInput").ap()
    bg_d = nc.dram_tensor("bg", [E, 1], F32, kind="ExternalInput").ap()
    wet_d = nc.dram_tensor("wet", [E, D, D], F32, kind="ExternalInput").ap()
    be_d = nc.dram_tensor("be", [E, D], F32, kind="ExternalInput").ap()
    idx_d = nc.dram_tensor("idx2048", [P, TC // 16], I16, kind="ExternalInput").ap()
    out_d = nc.dram_tensor("out", [TC, D], F32, kind="ExternalOutput").ap()

    with tile.TileContext(nc) as tc:
        _body(tc, x_d, wgt_d, bg_d, wet_d, be_d, idx_d, out_d)

    nc.compile()
    return nc


def _body(tc, x_d, wgt_d, bg_d, wet_d, be_d, idx_d, out_d):
    nc = tc.nc

    from contextlib import ExitStack

    ctx = ExitStack()
    with ctx:
        persist = ctx.enter_context(tc.tile_pool(name="persist", bufs=1))
        small = ctx.enter_context(tc.tile_pool(name="small", bufs=1))
        psum_pool = ctx.enter_context(
            tc.tile_pool(name="psum", bufs=2, space="PSUM")
        )

        # ---- persistent tiles ----
        # chunk-major transposed activations: [P, 4 chunks, KT, 512]
        xt_hi = persist.tile([P, 4, KT, 512], BF16, tag="xt_hi")   # 4 MiB
        xt_lo = persist.tile([P, 4, KT, 512], BF16, tag="xt_lo")   # 4 MiB
        acc = persist.tile([P, NT, D], BF16, tag="acc")        # 4 MiB

        # small persistent data
        idx_sb = small.tile([P, TC // 16], I16, tag="idx")
        nc.sync.dma_start(idx_sb[:], idx_d[:])
        bg_sb = small.tile([E, 1], F32, tag="bg")
        nc.sync.dma_start(bg_sb[:], bg_d[:])
        be_sb = small.tile([E, D], BF16, tag="be")
        nc.gpsimd.dma_start(out=be_sb[:], in_=be_d[:])  # cast f32->bf16

        # Wg split planes: [128, KT, E] each; stacked stationary for pass1
        wgt_sb = small.tile([P, KT, E], F32, tag="wgt")
        nc.sync.dma_start(
            wgt_sb[:], wgt_d.rearrange("(kt p) e -> p kt e", p=P)
        )
        wg1 = small.tile([P, KT, 96], BF16, tag="wg1")  # [hi|mid|lo] @32
        nc.vector.memset(wg1[:], 0.0)
        wg_hi = wg1[:, :, 0:E]
        wg_mid = wg1[:, :, 32 : 32 + E]
        wg_lo = wg1[:, :, 64 : 64 + E]
        wtmp = small.tile([P, KT, E], F32, tag="wtmp")
        nc.vector.tensor_copy(wg_hi, wgt_sb[:])                     # hi = bf16(w)
        nc.vector.tensor_sub(wtmp[:], wgt_sb[:], wg_hi)             # r1 = w - hi
        nc.vector.tensor_copy(wg_mid, wtmp[:])                      # mid = bf16(r1)
        nc.vector.tensor_sub(wtmp[:], wtmp[:], wg_mid)              # r2 = r1 - mid
        nc.vector.tensor_copy(wg_lo, wtmp[:])                       # lo = bf16(r2)

        # ---- stage 1: load x, build hi/lo planes ----
        with tc.tile_pool(name="xtmp", bufs=1) as xtmp:
            x_hi = xtmp.tile([P, NT, D], BF16, tag="x_hi")
            x_lo = xtmp.tile([P, NT, D], BF16, tag="x_lo")
            with tc.tile_pool(name="xload", bufs=3) as xload:
                xr = x_d.rearrange("(i p) d -> p i d", p=P)
                for i in range(NT):
                    xf = xload.tile([P, D], F32, tag="xf")
                    nc.sync.dma_start(xf[:], xr[:, i, :])
                    nc.vector.tensor_copy(x_hi[:, i, :], xf[:])
                    nc.vector.tensor_sub(
                        x_lo[:, i, :], xf[:], x_hi[:, i, :]
                    )

            # ---- stage 2: transpose-gather to [d, tok] (identity idx) ----
            # Chunked: keeps per-op descriptor count under the SWDGE ring
            # size and spreads chunks across DMA engines.
            GCH = 512
            for src, dst in ((x_hi, xt_hi), (x_lo, xt_lo)):
                for g in range(TC // GCH):
                    nc.gpsimd.dma_gather(
                        out_ap=dst[:, g, :, :],
                        in_ap=src[:],
                        idxs_ap=idx_sb[:, bass.ts(g, GCH // 16)],
                        num_idxs=GCH,
                        num_idxs_reg=GCH,
                        elem_size=D,
                        transpose=True,
                        sbuf_tokens_per_rank=P,
                        sbuf_free_dim_per_rank=D * 2,
                    )

        # ---- stage 3: gate logits (4-term split-bf16, fp32-exact) ----
        # pass1: lhsT = [wg_hi|wg_mid|wg_lo] (m=24), rhs = xt_hi
        # pass2: lhsT = wg_hi (m=8),           rhs = xt_lo
        logits_t = small.tile([32, TC], F32, tag="logits_t")
        nc.vector.memset(logits_t[:], 0.0)
        for tch in range(TC // 512):
            ps1 = psum_pool.tile([96, 512], F32, tag="ps_g1")
            ps2 = psum_pool.tile([E, 512], F32, tag="ps_g2")
            for kt in range(KT):
                nc.tensor.matmul(
                    ps1[:],
                    wg1[:, kt, :],
                    xt_hi[:, tch, kt, :],
                    start=(kt == 0),
                    stop=(kt == KT - 1),
                )
            for kt in range(KT):
                nc.tensor.matmul(
                    ps2[:],
                    wg1[:, kt, 0:E],
                    xt_lo[:, tch, kt, :],
                    start=(kt == 0),
                    stop=(kt == KT - 1),
                )
            # logitsT = hi + mid + lo + lo_x + bg
            # (DVE may read at most one PSUM operand per instruction)
            t1 = small.tile([E, 512], F32, tag="g_t1")
            nc.vector.tensor_copy(t1[:], ps1[0:E, :])
            nc.vector.tensor_add(t1[:], t1[:], ps1[32 : 32 + E, :])
            nc.vector.tensor_add(t1[:], t1[:], ps1[64 : 64 + E, :])
            nc.vector.tensor_add(t1[:], t1[:], ps2[:])
            nc.vector.tensor_scalar(
                logits_t[0:E, bass.ts(tch, 512)],
                t1[:],
                bg_sb[:],
                None,
                ALU.add,
            )

        # ---- stage 4: top-2 + softmax + combine ----
        mv = small.tile([P, NT, 8], F32, tag="mv")
        mi = small.tile([P, NT, 8], U32, tag="mi")
        lg = small.tile([P, NT, 32], F32, tag="lg")
        for j in range(NT):
            transpose_blocks(
                nc, lg[:, j, :], logits_t[:, bass.ts(j, P)], 32, P
            )
            nc.vector.max_with_indices(
                mv[:, j, :], mi[:, j, :], lg[:, j, 0:8]
            )
        # softmax over (v0, v1): p2 = 1/(1+exp(v0-v1)... ) careful:
        # d = v1 - v0 (<=0); e = exp(d); p0 = 1/(1+e); p1 = 1 - p0 = e*p0
        pd = small.tile([P, NT], F32, tag="pd")
        pe = small.tile([P, NT], F32, tag="pe")
        p0 = small.tile([P, NT], F32, tag="p0")
        p1 = small.tile([P, NT], F32, tag="p1")
        nc.vector.tensor_sub(pd[:], mv[:, :, 1], mv[:, :, 0])
        nc.scalar.activation(pe[:], pd[:], ACT_EXP)
        nc.vector.tensor_scalar_add(pd[:], pe[:], 1.0)
        nc.vector.reciprocal(p0[:], pd[:])
        nc.vector.tensor_mul(p1[:], pe[:], p0[:])

        # combine weights c[t, e] = p0*(i0==e) + p1*(i1==e)
        comb = small.tile([P, NT, E], F32, tag="comb")
        eq = small.tile([P, NT], F32, tag="eq")
        eq2 = small.tile([P, NT], F32, tag="eq2")
        for e in range(E):
            nc.vector.tensor_scalar(
                eq[:], mi[:, :, 0], float(e), None, ALU.is_equal
            )
            nc.vector.tensor_mul(eq[:], eq[:], p0[:])
            nc.vector.tensor_scalar(
                eq2[:], mi[:, :, 1], float(e), None, ALU.is_equal
            )
            nc.vector.tensor_mul(eq2[:], eq2[:], p1[:])
            nc.vector.tensor_add(comb[:, :, e], eq[:], eq2[:])

        # combine transposed (for bias matmul): cT [E, TC] bf16
        combt = small.tile([32, TC], BF16, tag="combt")
        ct32 = small.tile([32, TC], F32, tag="ct32")
        for j in range(NT):
            cpad = small.tile([P, 32], F32, tag="cpad")
            nc.vector.memset(cpad[:, E:32], 0.0)
            nc.vector.tensor_copy(cpad[:, 0:E], comb[:, j, :])
            transpose_blocks(nc, ct32[:, bass.ts(j, P)], cpad[:], P, 32)
        nc.vector.tensor_copy(combt[:], ct32[:])

        # ---- stage 5: acc init = combine @ be (bias term) ----
        for j in range(NT):
            for q in range(NQ):
                psb = psum_pool.tile([P, 512], F32, tag="ps_b")
                nc.tensor.matmul(
                    psb[:],
                    combt[0:E, bass.ts(j, P)],
                    be_sb[:, bass.ts(q, 512)],
                    start=True,
                    stop=True,
                )
                nc.vector.tensor_copy(acc[:, j, bass.ts(q, 512)], psb[:])

        # ---- stage 6: dense FFN with combine-weighted accumulation ----
        with tc.tile_pool(name="wpool", bufs=3) as wpool:
            for e in range(E):
                we_e = wpool.tile([P, KT, D], BF16, tag="we")
                nc.gpsimd.dma_start(
                    out=we_e[:],
                    in_=wet_d[e].rearrange("(kt p) o -> p kt o", p=P),
                )
                for j in range(NT):
                    for q in range(NQ):
                        ps = psum_pool.tile([P, 512], F32, tag="ps_f")
                        for kt in range(KT):
                            nc.tensor.matmul(
                                ps[:],
                                xt_hi[:, j // 4, kt, bass.ts(j % 4, P)],
                                we_e[:, kt, bass.ts(q, 512)],
                                start=(kt == 0),
                                stop=(kt == KT - 1),
                            )
                        # acc += c[:, j, e] * ps
                        sc = small.tile([P, 512], F32, tag="sc")
                        nc.vector.tensor_scalar(
                            sc[:], ps[:], comb[:, j, e : e + 1], None, ALU.mult
                        )
                        nc.vector.tensor_add(
                            acc[:, j, bass.ts(q, 512)],
                            acc[:, j, bass.ts(q, 512)],
                            sc[:],
                        )

        # ---- stage 7: store (bf16 -> f32 cast in DMA) ----
        nc.gpsimd.dma_start(
            out=out_d.rearrange("(i p) d -> p i d", p=P), in_=acc[:]
        )


_CACHED = None


def _marshal(x, Wg, bg, We, be):
    """Host-side input marshalling: shard x, transpose weights, index table."""
    xs = np.ascontiguousarray(x, dtype=np.float32)
    wgt = np.ascontiguousarray(np.asarray(Wg, dtype=np.float32).T)
    bgc = np.ascontiguousarray(np.asarray(bg, dtype=np.float32).reshape(E, 1))
    wet = np.ascontiguousarray(
        np.asarray(We, dtype=np.float32).transpose(0, 2, 1)
    )
    bec = np.ascontiguousarray(np.asarray(be, dtype=np.float32))
    idx = wrap16(np.arange(TC, dtype=np.int16)).astype(np.int16)
    in_maps = []
    for c in range(NCORES):
        shard = np.ascontiguousarray(xs[c * TC : (c + 1) * TC])
        in_maps.append(
            {
                "x": shard,
                "wgt": wgt,
                "bg": bgc,
                "wet": wet,
                "be": bec,
                "idx2048": idx,
            }
        )
    return in_maps


def kernel(x, Wg, bg, We, be):
    global _CACHED
    if _CACHED is None:
        _CACHED = build_program()
    nc = _CACHED
    in_maps = _marshal(x, Wg, bg, We, be)
    res = run_bass_kernel_spmd(nc, in_maps, list(range(NCORES)))
    out = np.concatenate(
        [res.results[c]["out"] for c in range(NCORES)], axis=0
    )
    return out.astype(np.float32)


if __name__ == "__main__":
    # CoreSim smoke test on one core's shard.
    from concourse.bass_interp import CoreSim

    rng = np.random.default_rng(0)
    x = rng.standard_normal((TC, D), dtype=np.float32)
    Wg = (rng.standard_normal((E, D)) / np.sqrt(D)).astype(np.float32)
    bg = np.zeros((E,), np.float32)
    We = (rng.standard_normal((E, D, D)) / np.sqrt(D)).astype(np.float32)
    be = (rng.standard_normal((E, D)) * 0.01).astype(np.float32)

    nc = build_program()
    in_maps = _marshal(
        np.tile(x, (NCORES, 1))[: T_FULL], Wg, bg, We, be
    )
    sim = CoreSim(nc)
    for k, v in in_maps[0].items():
        sim.tensor(k)[:] = v
    sim.simulate()
    got = sim.tensor("out").copy()

    # numpy reference (matching fp32 semantics closely enough for sanity)
    logits = x @ Wg.T + bg
    order = np.argsort(-logits, axis=1, kind="stable")[:, :TOPK]
    tv = np.take_along_axis(logits, order, axis=1)
    pm = np.exp(tv - tv.max(axis=1, keepdims=True))
    pm = pm / pm.sum(axis=1, keepdims=True)
    ref = np.zeros((TC, D), np.float32)
    for k in range(TOPK):
        eidx = order[:, k]
        ref += pm[:, k : k + 1] * (
            np.einsum("td,tod->to", x, We[eidx]) + be[eidx]
        )
    err = np.abs(got - ref)
    scale = np.abs(ref).max()
    print("absmax err:", err.max(), "scale:", scale, "rel:", err.max() / scale)


# revision 12
# speedup vs baseline: 151.5826x; 151.5826x over previous
"""MoE routing kernel for Trainium2 (8 NeuronCores, SPMD data-parallel).

Problem: T=16384 tokens, D=1024, E=8 experts, top-2 routing.
  gate_logits = x @ Wg.T + bg ; top2 -> softmax -> combine
  out[t] = sum_e combine[t,e] * (x[t] @ We[e].T + be[e])

Sharding: data-parallel. Each core owns 2048 tokens, experts replicated.
Host only slices/transposes inputs and concatenates the 8 output shards.

Device pipeline (per core):
  1. x shard -> SBUF, split into bf16 (hi, lo) planes.
  2. transpose-gather (identity indices) -> xT_hi / xT_lo  [d-part, tok].
  3. fp32-accurate gate logits via 4-term split-bf16 matmul, +bg.
  4. top-2 + softmax on DVE (max8 / max_index), combine weights.
  5. dense per-expert matmul (bf16) with combine-weighted accumulation,
     bias handled as init acc = combine @ be.
"""

import os
import sys
import numpy as np

sys.path.insert(0, "/opt/trn_rl_repo")

import concourse.bass as bass
import concourse.tile as tile
from concourse import bacc, mybir
from concourse.bass_utils import run_bass_kernel_spmd

F32 = mybir.dt.float32
BF16 = mybir.dt.bfloat16
I16 = mybir.dt.int16
U32 = mybir.dt.uint32
ALU = mybir.AluOpType
ACT_EXP = mybir.ActivationFunctionType.Exp

T_FULL = 16384
D = 1024
E = 8
TOPK = 2
NCORES = 8
TC = T_FULL // NCORES     # 2048 tokens per core
P = 128
NT = TC // P              # 16 token tiles
KT = D // P               # 8 contraction tiles
NQ = D // 512             # 2 output column halves


def wrap16(vals: np.ndarray) -> np.ndarray:
    """Pack a 1-D slot array into the [128, n/16] 'wrapped in 16 partitions,
    replicated across 8 cores' SBUF layout used by dma_gather index inputs."""
    n = vals.shape[0]
    assert n % 16 == 0
    w = vals.reshape(n // 16, 16).T        # [16, n/16]
    return np.tile(w, (8, 1)).copy()       # [128, n/16]


def transpose_blocks(nc, out_ap, in_ap, rows, cols):
    """out[c, r] = in[r, c] via DVE 32x32 block transposes.

    in_ap indexable as [rows, cols], out_ap as [cols, rows]; both SBUF,
    rows/cols multiples of 32. Slicing must yield [32, 32] APs.
    """
    for rb in range(rows // 32):
        for cb in range(cols // 32):
            nc.vector.transpose(
                out_ap[cb * 32 : (cb + 1) * 32, rb * 32 : (rb + 1) * 32],
                in_ap[rb * 32 : (rb + 1) * 32, cb * 32 : (cb + 1) * 32],
            )


def build_program():
    nc = bacc.Bacc(
        "TRN2", target_bir_lowering=False, debug=False, num_devices=NCORES
    )

    x_d = nc.dram_tensor("x", [TC, D], F32, kind="ExternalInput").ap()
    wgt_d = nc.dram_tensor("wgt", [D, E], F32, kind="ExternalInput").ap()
    bg_d = nc.dram_tensor("bg", [E, 1], F32, kind="ExternalInput").ap()
    wet_d = nc.dram_tensor("wet", [E, D, D], F32, kind="ExternalInput").ap()
    be_d = nc.dram_tensor("be", [E, D], F32, kind="ExternalInput").ap()
    idx_d = nc.dram_tensor("idx2048", [P, TC // 16], I16, kind="ExternalInput").ap()
    out_d = nc.dram_tensor("out", [TC, D], F32, kind="ExternalOutput").ap()

    with tile.TileContext(nc) as tc:
        _body(tc, x_d, wgt_d, bg_d, wet_d, be_d, idx_d, out_d)

    nc.compile()
    return nc


def _body(tc, x_d, wgt_d, bg_d, wet_d, be_d, idx_d, out_d):
    nc = tc.nc

    from contextlib import ExitStack

    ctx = ExitStack()
    with ctx:
        persist = ctx.enter_context(tc.tile_pool(name="persist", bufs=1))
        small = ctx.enter_context(tc.tile_pool(name="small", bufs=1))
        psum_pool = ctx.enter_context(
            tc.tile_pool(name="psum", bufs=2, space="PSUM")
        )

        # ---- persistent tiles ----
        # chunk-major transposed activations: [P, 4 chunks, KT, 512]
        xt_hi = persist.tile([P, 4, KT, 512], BF16, tag="xt_hi")   # 4 MiB
        xt_lo = persist.tile([P, 4, KT, 512], BF16, tag="xt_lo")   # 4 MiB
        acc = persist.tile([P, NT, D], BF16, tag="acc")        # 4 MiB

        # small persistent data
        idx_sb = small.tile([P, TC // 16], I16, tag="idx")
        nc.sync.dma_start(idx_sb[:], idx_d[:])
        bg_sb = small.tile([E, 1], F32, tag="bg")
        nc.sync.dma_start(bg_sb[:], bg_d[:])
        be_sb = small.tile([E, D], BF16, tag="be")
        nc.gpsimd.dma_start(out=be_sb[:], in_=be_d[:])  # cast f32->bf16

        # Wg split planes: [128, KT, E] each; stacked stationary for pass1
        wgt_sb = small.tile([P, KT, E], F32, tag="wgt")
        nc.sync.dma_start(
            wgt_sb[:], wgt_d.rearrange("(kt p) e -> p kt e", p=P)
        )
        wg1 = small.tile([P, KT, 96], BF16, tag="wg1")  # [hi|mid|lo] @32
        nc.vector.memset(wg1[:], 0.0)
        wg_hi = wg1[:, :, 0:E]
        wg_mid = wg1[:, :, 32 : 32 + E]
        wg_lo = wg1[:, :, 64 : 64 + E]
        wtmp = small.tile([P, KT, E], F32, tag="wtmp")
        nc.vector.tensor_copy(wg_hi, wgt_sb[:])                     # hi = bf16(w)
        nc.vector.tensor_sub(wtmp[:], wgt_sb[:], wg_hi)             # r1 = w - hi
        nc.vector.tensor_copy(wg_mid, wtmp[:])                      # mid = bf16(r1)
        nc.vector.tensor_sub(wtmp[:], wtmp[:], wg_mid)              # r2 = r1 - mid
        nc.vector.tensor_copy(wg_lo, wtmp[:])                       # lo = bf16(r2)

        # ---- stage 1: load x, build hi/lo planes ----
        with tc.tile_pool(name="xtmp", bufs=1) as xtmp:
            x_hi = xtmp.tile([P, NT, D], BF16, tag="x_hi")
            x_lo = xtmp.tile([P, NT, D], BF16, tag="x_lo")
            with tc.tile_pool(name="xload", bufs=3) as xload:
                xr = x_d.rearrange("(i p) d -> p i d", p=P)
                for i in range(NT):
                    xf = xload.tile([P, D], F32, tag="xf")
                    nc.sync.dma_start(xf[:], xr[:, i, :])
                    nc.vector.tensor_copy(x_hi[:, i, :], xf[:])
                    nc.vector.tensor_sub(
                        x_lo[:, i, :], xf[:], x_hi[:, i, :]
                    )

            # ---- stage 2: transpose-gather to [d, tok] (identity idx) ----
            # Chunked: keeps per-op descriptor count under the SWDGE ring
            # size and spreads chunks across DMA engines.
            GCH = 512
            for src, dst in ((x_hi, xt_hi), (x_lo, xt_lo)):
                for g in range(TC // GCH):
                    nc.gpsimd.dma_gather(
                        out_ap=dst[:, g, :, :],
                        in_ap=src[:],
                        idxs_ap=idx_sb[:, bass.ts(g, GCH // 16)],
                        num_idxs=GCH,
                        num_idxs_reg=GCH,
                        elem_size=D,
                        transpose=True,
                        sbuf_tokens_per_rank=P,
                        sbuf_free_dim_per_rank=D * 2,
                    )

        # ---- stage 3: gate logits (4-term split-bf16, fp32-exact) ----
        # pass1: lhsT = [wg_hi|wg_mid|wg_lo] (m=24), rhs = xt_hi
        # pass2: lhsT = wg_hi (m=8),           rhs = xt_lo
        logits_t = small.tile([32, TC], F32, tag="logits_t")
        nc.vector.memset(logits_t[:], 0.0)
        for tch in range(TC // 512):
            ps1 = psum_pool.tile([96, 512], F32, tag="ps_g1")
            ps2 = psum_pool.tile([E, 512], F32, tag="ps_g2")
            for kt in range(KT):
                nc.tensor.matmul(
                    ps1[:],
                    wg1[:, kt, :],
                    xt_hi[:, tch, kt, :],
                    start=(kt == 0),
                    stop=(kt == KT - 1),
                )
            for kt in range(KT):
                nc.tensor.matmul(
                    ps2[:],
                    wg1[:, kt, 0:E],
                    xt_lo[:, tch, kt, :],
                    start=(kt == 0),
                    stop=(kt == KT - 1),
                )
            # logitsT = hi + mid + lo + lo_x + bg
            # (DVE may read at most one PSUM operand per instruction)
            t1 = small.tile([E, 512], F32, tag="g_t1")
            nc.vector.tensor_copy(t1[:], ps1[0:E, :])
            nc.vector.tensor_add(t1[:], t1[:], ps1[32 : 32 + E, :])
            nc.vector.tensor_add(t1[:], t1[:], ps1[64 : 64 + E, :])
            nc.vector.tensor_add(t1[:], t1[:], ps2[:])
            nc.vector.tensor_scalar(
                logits_t[0:E, bass.ts(tch, 512)],
                t1[:],
                bg_sb[:],
                None,
                ALU.add,
            )

        # ---- stage 4: top-2 + softmax + combine ----
        mv = small.tile([P, NT, 8], F32, tag="mv")
        mi = small.tile([P, NT, 8], U32, tag="mi")
        lg = small.tile([P, NT, 32], F32, tag="lg")
        for j in range(NT):
            transpose_blocks(
                nc, lg[:, j, :], logits_t[:, bass.ts(j, P)], 32, P
            )
            nc.vector.max_with_indices(
                mv[:, j, :], mi[:, j, :], lg[:, j, 0:8]
            )
        # softmax over (v0, v1): p2 = 1/(1+exp(v0-v1)... ) careful:
        # d = v1 - v0 (<=0); e = exp(d); p0 = 1/(1+e); p1 = 1 - p0 = e*p0
        pd = small.tile([P, NT], F32, tag="pd")
        pe = small.tile([P, NT], F32, tag="pe")
        p0 = small.tile([P, NT], F32, tag="p0")
        p1 = small.tile([P, NT], F32, tag="p1")
        nc.vector.tensor_sub(pd[:], mv[:, :, 1], mv[:, :, 0])
        nc.scalar.activation(pe[:], pd[:], ACT_EXP)
        nc.vector.tensor_scalar_add(pd[:], pe[:], 1.0)
        nc.vector.reciprocal(p0[:], pd[:])
        nc.vector.tensor_mul(p1[:], pe[:], p0[:])

        # combine weights c[t, e] = p0*(i0==e) + p1*(i1==e)
        comb = small.tile([P, NT, E], F32, tag="comb")
        eq = small.tile([P, NT], F32, tag="eq")
        eq2 = small.tile([P, NT], F32, tag="eq2")
        for e in range(E):
            nc.vector.tensor_scalar(
                eq[:], mi[:, :, 0], float(e), None, ALU.is_equal
            )
            nc.vector.tensor_mul(eq[:], eq[:], p0[:])
            nc.vector.tensor_scalar(
                eq2[:], mi[:, :, 1], float(e), None, ALU.is_equal
            )
            nc.vector.tensor_mul(eq2[:], eq2[:], p1[:])
            nc.vector.tensor_add(comb[:, :, e], eq[:], eq2[:])

        # combine transposed (for bias matmul): cT [E, TC] bf16
        combt = small.tile([32, TC], BF16, tag="combt")
        ct32 = small.tile([32, TC], F32, tag="ct32")
        for j in range(NT):
            cpad = small.tile([P, 32], F32, tag="cpad")
            nc.vector.memset(cpad[:, E:32], 0.0)
            nc.vector.tensor_copy(cpad[:, 0:E], comb[:, j, :])
            transpose_blocks(nc, ct32[:, bass.ts(j, P)], cpad[:], P, 32)
        nc.vector.tensor_copy(combt[:], ct32[:])

        # ---- stage 5: acc init = combine @ be (bias term) ----
        for j in range(NT):
            for q in range(NQ):
                psb = psum_pool.tile([P, 512], F32, tag="ps_b")
                nc.tensor.matmul(
                    psb[:],
                    combt[0:E, bass.ts(j, P)],
                    be_sb[:, bass.ts(q, 512)],
                    start=True,
                    stop=True,
                )
                nc.vector.tensor_copy(acc[:, j, bass.ts(q, 512)], psb[:])

        # ---- stage 6: dense FFN with combine-weighted accumulation ----
        with tc.tile_pool(name="wpool", bufs=3) as wpool:
            for e in range(E):
                we_e = wpool.tile([P, KT, D], BF16, tag="we")
                nc.gpsimd.dma_start(
                    out=we_e[:],
                    in_=wet_d[e].rearrange("(kt p) o -> p kt o", p=P),
                )
                for j in range(NT):
                    for q in range(NQ):
                        ps = psum_pool.tile([P, 512], F32, tag="ps_f")
                        for kt in range(KT):
                            nc.tensor.matmul(
                                ps[:],
                                xt_hi[:, j // 4, kt, bass.ts(j % 4, P)],
                                we_e[:, kt, bass.ts(q, 512)],
                                start=(kt == 0),
                                stop=(kt == KT - 1),
                            )
                        # acc += c[:, j, e] * ps
                        sc = small.tile([P, 512], F32, tag="sc")
                        nc.vector.tensor_scalar(
                            sc[:], ps[:], comb[:, j, e : e + 1], None, ALU.mult
                        )
                        nc.vector.tensor_add(
                            acc[:, j, bass.ts(q, 512)],
                            acc[:, j, bass.ts(q, 512)],
                            sc[:],
                        )

        # ---- stage 7: store (bf16 -> f32 cast in DMA) ----
        nc.gpsimd.dma_start(
            out=out_d.rearrange("(i p) d -> p i d", p=P), in_=acc[:]
        )


_CACHED = None


class _Runner:
    """Cached PJRT executor: builds the shard_map-jitted NEFF wrapper once
    and reuses it (and donates previous outputs) across calls."""

    def __init__(self, nc):
        import jax
        from jax.sharding import Mesh, PartitionSpec
        from jax.experimental.shard_map import shard_map
        from concourse import mybir as _mb
        from concourse import bass2jax as _b2j

        _b2j.install_neuronx_cc_hook()
        self.nc = nc
        partition_name = (
            nc.partition_id_tensor.name if nc.partition_id_tensor else None
        )
        in_names, out_names, out_avals, zero_outs = [], [], [], []
        for alloc in nc.m.functions[0].allocations:
            if not isinstance(alloc, _mb.MemoryLocationSet):
                continue
            name = alloc.memorylocations[0].name
            if alloc.kind == "ExternalInput":
                if name != partition_name:
                    in_names.append(name)
            elif alloc.kind == "ExternalOutput":
                out_names.append(name)
                shape = tuple(alloc.tensor_shape)
                dtype = _mb.dt.np(alloc.dtype)
                out_avals.append(jax.core.ShapedArray(shape, dtype))
                zero_outs.append(np.zeros(shape, dtype))
        self.in_names = list(in_names)
        self.out_names = out_names
        self.out_avals = out_avals
        self.zero_outs = zero_outs
        n_params = len(in_names)
        n_outs = len(out_avals)
        all_in_names = list(in_names) + list(out_names)
        if partition_name is not None:
            all_in_names.append(partition_name)
        all_in_names = tuple(all_in_names)
        donate = tuple(range(n_params, n_params + n_outs))

        def _body(*args):
            operands = list(args)
            if partition_name is not None:
                operands.append(_b2j.partition_id_tensor())
            outs = _b2j._bass_exec_p.bind(
                *operands,
                out_avals=tuple(out_avals),
                in_names=all_in_names,
                out_names=tuple(out_names),
                lowering_input_output_aliases=(),
                sim_require_finite=True,
                sim_require_nnan=True,
                nc=nc,
            )
            return tuple(outs)

        devices = jax.devices()[:NCORES]
        self.mesh = Mesh(np.asarray(devices), ("core",))
        in_specs = (PartitionSpec("core"),) * (n_params + n_outs)
        out_specs = (PartitionSpec("core"),) * n_outs
        self.fn = jax.jit(
            shard_map(
                _body,
                mesh=self.mesh,
                in_specs=in_specs,
                out_specs=out_specs,
                check_rep=False,
            ),
            donate_argnums=donate,
            keep_unused=True,
        )
        self._last_outs = None

    def concat_inputs(self, in_maps):
        return [
            np.concatenate([np.asarray(m[n]) for m in in_maps], axis=0)
            for n in self.in_names
        ]

    def run(self, concat_in):
        import jax

        if self._last_outs is None:
            douts = [
                np.zeros((NCORES * z.shape[0], *z.shape[1:]), z.dtype)
                for z in self.zero_outs
            ]
        else:
            douts = self._last_outs
        outs = self.fn(*concat_in, *douts)
        outs = list(outs)
        jax.block_until_ready(outs)
        self._last_outs = outs
        return outs

    def split(self, outs):
        res = []
        for c in range(NCORES):
            res.append(
                {
                    n: np.asarray(outs[i]).reshape(
                        NCORES, *self.out_avals[i].shape
                    )[c]
                    for i, n in enumerate(self.out_names)
                }
            )
        return res


_RUNNER = None


def get_runner():
    global _CACHED, _RUNNER
    if _CACHED is None:
        _CACHED = build_program()
    if _RUNNER is None:
        _RUNNER = _Runner(_CACHED)
    return _RUNNER


def _marshal(x, Wg, bg, We, be):
    """Host-side input marshalling: shard x, transpose weights, index table."""
    xs = np.ascontiguousarray(x, dtype=np.float32)
    wgt = np.ascontiguousarray(np.asarray(Wg, dtype=np.float32).T)
    bgc = np.ascontiguousarray(np.asarray(bg, dtype=np.float32).reshape(E, 1))
    wet = np.ascontiguousarray(
        np.asarray(We, dtype=np.float32).transpose(0, 2, 1)
    )
    bec = np.ascontiguousarray(np.asarray(be, dtype=np.float32))
    idx = wrap16(np.arange(TC, dtype=np.int16)).astype(np.int16)
    in_maps = []
    for c in range(NCORES):
        shard = np.ascontiguousarray(xs[c * TC : (c + 1) * TC])
        in_maps.append(
            {
                "x": shard,
                "wgt": wgt,
                "bg": bgc,
                "wet": wet,
                "be": bec,
                "idx2048": idx,
            }
        )
    return in_maps


def kernel(x, Wg, bg, We, be):
    r = get_runner()
    in_maps = _marshal(x, Wg, bg, We, be)
    outs = r.run(r.concat_inputs(in_maps))
    # concat over cores is exactly the full [T, D] output (axis-0 shards)
    out = np.asarray(outs[r.out_names.index("out")])
    return out.astype(np.float32)


if __name__ == "__main__":
    # CoreSim smoke test on one core's shard.
    from concourse.bass_interp import CoreSim

    rng = np.random.default_rng(0)
    x = rng.standard_normal((TC, D), dtype=np.float32)
    Wg = (rng.standard_normal((E, D)) / np.sqrt(D)).astype(np.float32)
    bg = np.zeros((E,), np.float32)
    We = (rng.standard_normal((E, D, D)) / np.sqrt(D)).astype(np.float32)
    be = (rng.standard_normal((E, D)) * 0.01).astype(np.float32)

    nc = build_program()
    in_maps = _marshal(
        np.tile(x, (NCORES, 1))[: T_FULL], Wg, bg, We, be
    )
    sim = CoreSim(nc)
    for k, v in in_maps[0].items():
        sim.tensor(k)[:] = v
    sim.simulate()
    got = sim.tensor("out").copy()

    # numpy reference (matching fp32 semantics closely enough for sanity)
    logits = x @ Wg.T + bg
    order = np.argsort(-logits, axis=1, kind="stable")[:, :TOPK]
    tv = np.take_along_axis(logits, order, axis=1)
    pm = np.exp(tv - tv.max(axis=1, keepdims=True))
    pm = pm / pm.sum(axis=1, keepdims=True)
    ref = np.zeros((TC, D), np.float32)
    for k in range(TOPK):
        eidx = order[:, k]
        ref += pm[:, k : k + 1] * (
            np.einsum("td,tod->to", x, We[eidx]) + be[eidx]
        )
    err = np.abs(got - ref)
    scale = np.abs(ref).max()
    print("absmax err:", err.max(), "scale:", scale, "rel:", err.max() / scale)
